# revision 20
# baseline (speedup 1.0000x reference)
"""Trainium2 Bass kernel for masked graph-convolution interaction.

Math (reference):
    wf = node_features @ weight                              # [N, D]
    T[i,d,j] = wf[i,d] * wf[j,d] * mh[i,j]
    S[a,d,j] = sum_i adj[a,i] * T[i,d,j]
    out[a,d] = sum_j S[a,d,j] * mf[a,j] / ncnt[a]^2

fp8 scheme (mean-centered so e4m3 quantization error stays ~1e-2):
    mh = 0.5 + mh',  adj = 0.5 + adj',  mfs = mf / ncnt^2 (folded on host)
    X'_a[i,d] = adj'[a,i] * wf[i,d]                 (fp8, DVE)
    Y'_a = mh'8^T @ X'8_a                           (PE, fp8 DoubleRow = 2x)
    out[a,d] = sum_j mfs[a,j] wf[j,d] Y'_a[j,d] + C[a,d]
    C = 0.5*(adj@wf) .* (mfs@wf) + 0.5*mfs@(wf .* (mh'^T@wf))   (exact, tiny)

wf is tiny (1024x128) and replicated, so it is computed on the host (the
sharding hint replicates wf) and shipped pre-laid-out in bf16, along with
its 4x-replicated copy used by the Z elementwise stage.

Pipeline (per 4-row group b): X' is produced TWO groups ahead on DVE so the
PE never waits on it; Y' drains via ACT (f32->bf16); Z = Ybf .* wf on DVE
(bf16 2x); the j-contraction runs as 4-way col-tiled PE matvecs batched two
groups at a time, LAG groups behind; C is pre-shuffled once into the matvec
output layout by a SBUF->SBUF DMA so no one-hot matmuls are needed; output
rows DMA out per 4-group quarter.  Group 0 runs icp-major so its first
matmuls only need the first X' chunks + the first mh8 quarter.  Odd groups
iterate (jc, icp) in reverse so each group starts with the stationary weight
its predecessor finished with (no weight reload across the matvec blocks).

Sharding: row-split of a across 8 cores (128 rows each); mh / wf replicated.
"""

import numpy as np

N = 1024
DIN = 256
DOUT = 128
NCORES = 8
ROWS = N // NCORES  # 128 output rows per core
P = 128
IC = N // P  # 8 chunks over i
ICP = IC // 2  # 4 DoubleRow pairs
JC = N // P  # 8 chunks over j
G4 = 4  # rows per group
NG = ROWS // G4  # 32 groups per core
LAG = 2  # matvec runs >= LAG groups behind the main matmul

_DTYPE = "fp8_doublerow"  # informational (test.py prints it)

_CACHE = {}


def _build():
    import concourse.bass as bass
    import concourse.tile as tile
    from concourse import bacc, mybir
    from concourse._compat import axon_active

    f32 = mybir.dt.float32
    bf = mybir.dt.bfloat16
    f8 = mybir.dt.float8e4
    Copy = mybir.ActivationFunctionType.Copy
    DR = mybir.MatmulPerfMode.DoubleRow

    nc = bacc.Bacc(
        "TRN2",
        target_bir_lowering=False,
        debug=not axon_active(),
        num_devices=NCORES,
    )

    # host-prearranged [128, rowbytes] layouts (contiguous DMA descriptors)
    mh8_d = nc.dram_tensor("mh8", [P, IC * N], f8, kind="ExternalInput").ap()
    adjTc_d = nc.dram_tensor("adjTc", [P, IC * ROWS], bf, kind="ExternalInput").ap()
    mfT_d = nc.dram_tensor("mfT", [P, JC * ROWS], bf, kind="ExternalInput").ap()
    wfb_d = nc.dram_tensor("wfb", [P, IC * DOUT], bf, kind="ExternalInput").ap()
    wf4_d = nc.dram_tensor("wf4", [P, JC * 512], bf, kind="ExternalInput").ap()
    out_d = nc.dram_tensor("out", [ROWS, DOUT], f32, kind="ExternalOutput").ap()

    with tile.TileContext(nc) as tc:
        with (
            tc.tile_pool(name="const", bufs=1) as cpool,
            tc.tile_pool(name="x", bufs=3) as xpool,
            tc.tile_pool(name="y", bufs=3) as ypool,
            tc.tile_pool(name="z", bufs=5) as zpool,
            tc.tile_pool(name="py", bufs=3, space="PSUM") as pypool,
            tc.tile_pool(name="pout", bufs=2, space="PSUM") as popool,
        ):
            # ---- resident tiles + input DMA, ordered by first consumer ----
            wfb_sb = cpool.tile([P, N], bf, tag="wfb")
            mh8_sb = cpool.tile([P, IC * N], f8, tag="mh8")
            adjTc_sb = cpool.tile([P, N], bf, tag="adjTc")
            mfT_sb = cpool.tile([P, N], bf, tag="mfT")
            wf4_sb = cpool.tile([P, JC * 512], bf, tag="wf4")

            nc.sync.dma_start(adjTc_sb[:, : N // 2], adjTc_d[:, : N // 2])
            nc.sync.dma_start(wfb_sb[:], wfb_d)
            nc.sync.dma_start(mh8_sb[:, : 2 * N], mh8_d[:, : 2 * N])
            nc.sync.dma_start(adjTc_sb[:, N // 2 :], adjTc_d[:, N // 2 :])
            for icp in range(1, ICP):
                nc.sync.dma_start(
                    mh8_sb[:, 2 * icp * N : (2 * icp + 2) * N],
                    mh8_d[:, 2 * icp * N : (2 * icp + 2) * N],
                )
            nc.sync.dma_start(wf4_sb[:], wf4_d)
            nc.sync.dma_start(mfT_sb[:], mfT_d)


            # broadcast views for the X' formation (shared across groups)
            wf_bc = (
                wfb_sb[:]
                .rearrange("p (ic d) -> p ic d", ic=IC)
                .unsqueeze(2)
                .broadcast_to([P, IC, G4, DOUT])
            )
            adj_r = adjTc_sb[:].rearrange("p (ic r) -> p ic r", ic=IC)

            def emit_x_chunked(b, x_t):
                # group 0 paces the prologue: per-chunk ops start as soon as
                # the wf / adjTc DMAs land
                for ic in range(IC):
                    wf_c = (
                        wfb_sb[:, ic * DOUT : (ic + 1) * DOUT]
                        .rearrange("p (i d) -> p i d", i=1)
                        .unsqueeze(2)
                        .broadcast_to([P, 1, G4, DOUT])
                    )
                    adj_c = (
                        adj_r[:, ic : ic + 1, b * G4 : (b + 1) * G4]
                        .unsqueeze(3)
                        .broadcast_to([P, 1, G4, DOUT])
                    )
                    x_c = x_t[:, ic * G4 * DOUT : (ic + 1) * G4 * DOUT].rearrange(
                        "p (i s d) -> p i s d", i=1, s=G4
                    )
                    nc.vector.tensor_mul(x_c, wf_c, adj_c)

            def emit_x(b, x_t):
                # X'[i,(s,d)] = adj'[a,i] * wf[i,d]  -> fp8, one wide DVE op
                adj_bc = (
                    adj_r[:, :, G4 * b : G4 * (b + 1)]
                    .unsqueeze(3)
                    .broadcast_to([P, IC, G4, DOUT])
                )
                x_v = x_t[:].rearrange("p (ic s d) -> p ic s d", ic=IC, s=G4)
                nc.vector.tensor_mul(x_v, wf_bc, adj_bc)

            x_tiles = {}
            x_tiles[0] = xpool.tile([P, IC * G4 * DOUT], f8, tag="X", name="x_t")
            emit_x_chunked(0, x_tiles[0])
            x_tiles[1] = xpool.tile([P, IC * G4 * DOUT], f8, tag="X", name="x_t")
            emit_x(1, x_tiles[1])

            # adjT = adjTc + 0.5 (recovers full adj for the S0 correction)
            adjT_sb = cpool.tile([P, N], bf, tag="adjT")
            nc.vector.tensor_scalar_add(adjT_sb[:], adjTc_sb[:], 0.5)

            # C in both layouts: [a, d] and the matvec-output layout
            # (partition 32s, col b*D+d  <-  row 4b+s)
            C_sb = cpool.tile([P, DOUT], bf, tag="C")
            C_shuf = cpool.tile([P, NG * DOUT], bf, tag="C_shuf")

            def emit_corrections():
                # Q'[j,d] = sum_i mh'[i,j] wf[i,d]  (fp8 lhsT x bf16 rhs)
                Q_sb = cpool.tile([P, N], f32, tag="Q")
                for jc in range(JC):
                    pq = pypool.tile([P, 1024], f32, tag="py")
                    for ic in range(IC):
                        nc.tensor.matmul(
                            pq[:, :DOUT],
                            lhsT=mh8_sb[:, ic * N + jc * P : ic * N + (jc + 1) * P],
                            rhs=wfb_sb[:, ic * DOUT : (ic + 1) * DOUT],
                            start=(ic == 0),
                            stop=(ic == IC - 1),
                        )
                    nc.scalar.activation(
                        Q_sb[:, jc * DOUT : (jc + 1) * DOUT], pq[:, :DOUT], Copy
                    )
                # Vq = wf .* Q'  (bf16)
                Vq_sb = cpool.tile([P, N], bf, tag="Vq")
                nc.vector.tensor_mul(Vq_sb[:], wfb_sb[:], Q_sb[:])

                # S0 = adj @ wf; mwf = mfs @ wf; G = mfs @ Vq   (bf16 matmuls)
                s0_sb = cpool.tile([P, DOUT], f32, tag="s0")
                mwf_sb = cpool.tile([P, DOUT], f32, tag="mwf")
                g_sb = cpool.tile([P, DOUT], f32, tag="g")
                for dst, lhs_tile, rhs_tile in (
                    (s0_sb, adjT_sb, wfb_sb),
                    (mwf_sb, mfT_sb, wfb_sb),
                    (g_sb, mfT_sb, Vq_sb),
                ):
                    ps = pypool.tile([P, 1024], f32, tag="py")
                    for c in range(N // P):
                        nc.tensor.matmul(
                            ps[:, :DOUT],
                            lhsT=lhs_tile[:, c * P : (c + 1) * P],
                            rhs=rhs_tile[:, c * DOUT : (c + 1) * DOUT],
                            start=(c == 0),
                            stop=(c == N // P - 1),
                        )
                    nc.vector.tensor_copy(dst[:], ps[:, :DOUT])

                # C = 0.5*(S0 .* mwf + G)   (bf16)
                tmp_sb = cpool.tile([P, DOUT], f32, tag="tmpC")
                nc.vector.tensor_mul(tmp_sb[:], s0_sb[:], mwf_sb[:])
                nc.vector.tensor_add(tmp_sb[:], tmp_sb[:], g_sb[:])
                nc.vector.tensor_scalar_mul(C_sb[:], tmp_sb[:], 0.5)
                # pre-shuffle C into matvec output layout (sb->sb DMA):
                # C_shuf[32s, b*D+d] = C[4b+s, d]
                for s in range(G4):
                    nc.sync.dma_start(
                        C_shuf[32 * s : 32 * s + 1, :], C_sb[s:P:G4, :]
                    )

            out_sb = cpool.tile([P, NG * DOUT], f32, tag="out_sb")

            def emit_mains(b, x_t, ybf, rev):
                # main matmul: Y'[j,(s,d)] accumulated over i-pairs (fp8 DR).
                # rev groups iterate backwards so the first stationary weight
                # equals the previous group's last (no reload at boundaries).
                for jh in (range(JC // 2 - 1, -1, -1) if rev else range(JC // 2)):
                    py = pypool.tile([P, 1024], f32, tag="py")
                    for jl in ((1, 0) if rev else (0, 1)):
                        jc = jh * 2 + jl
                        icps = range(ICP - 1, -1, -1) if rev else range(ICP)
                        first, last = (ICP - 1, 0) if rev else (0, ICP - 1)
                        for icp in icps:
                            lhsT3 = mh8_sb[
                                :, 2 * icp * N : (2 * icp + 2) * N
                            ].rearrange("p (k f) -> p k f", k=2)[
                                :, :, jc * P : (jc + 1) * P
                            ]
                            rhs3 = x_t[
                                :, 2 * icp * 512 : (2 * icp + 2) * 512
                            ].rearrange("p (k f) -> p k f", k=2)
                            nc.tensor.matmul(
                                py[:, jl * 512 : (jl + 1) * 512],
                                lhsT=lhsT3,
                                rhs=rhs3,
                                start=(icp == first),
                                stop=(icp == last),
                                perf_mode=DR,
                            )
                    # drain 2 banks at once on ACT (f32 -> bf16)
                    nc.scalar.activation(
                        ybf[:, jh * 1024 : (jh + 1) * 1024], py[:], Copy
                    )

            def emit_mains_icp_major(b, x_t, ybf):
                # group 0: icp-major so the first matmuls only need the first
                # X' chunks and the first mh8 pair (everything still in DMA)
                for half in range(2):
                    pys = [
                        pypool.tile([P, 1024], f32, tag="py", name="py0"),
                        pypool.tile([P, 1024], f32, tag="py", name="py1"),
                    ]
                    for icp in range(ICP):
                        lhsT_all = mh8_sb[
                            :, 2 * icp * N : (2 * icp + 2) * N
                        ].rearrange("p (k f) -> p k f", k=2)
                        rhs3 = x_t[
                            :, 2 * icp * 512 : (2 * icp + 2) * 512
                        ].rearrange("p (k f) -> p k f", k=2)
                        for jcl in range(4):
                            jc = 4 * half + jcl
                            nc.tensor.matmul(
                                pys[jcl // 2][:, (jcl % 2) * 512 : (jcl % 2 + 1) * 512],
                                lhsT=lhsT_all[:, :, jc * P : (jc + 1) * P],
                                rhs=rhs3,
                                start=(icp == 0),
                                stop=(icp == ICP - 1),
                                perf_mode=DR,
                            )
                    for k in range(2):
                        jh = 2 * half + k
                        nc.scalar.activation(
                            ybf[:, jh * 1024 : (jh + 1) * 1024], pys[k][:], Copy
                        )

            def emit_z(ybf, rev=False):
                # Z = Ybf .* wf (bf16, DVE 2x)
                z_t = zpool.tile([P, JC * 512], bf, tag="Z")
                for h in ((1, 0) if rev else (0, 1)):
                    nc.vector.tensor_mul(
                        z_t[:, h * 2048 : (h + 1) * 2048],
                        ybf[:, h * 2048 : (h + 1) * 2048],
                        wf4_sb[:, h * 2048 : (h + 1) * 2048],
                    )
                return z_t

            def emit_matvec(b, z_t):
                # matvec: out[a,d] = sum_j mfs[a,j] Z[j,(s,d)]; 2-way col-tiled
                # with 2 rows per tile.  BUT each tile's two rows need rhs
                # slices from different s -> keep 4 separate MMs per jc when
                # the rhs differs; 2-row batching only works for the pair
                # whose rhs is shared, which it is not.  So: 4-way col-tiled
                # at 32-granularity, rows at partitions 0/32/64/96.
                pout = popool.tile([P, 512], f32, tag="pout")
                for jc in range(JC):
                    for s in range(G4):
                        a = b * G4 + s
                        nc.tensor.matmul(
                            pout[32 * s : 32 * s + 1, :DOUT],
                            lhsT=mfT_sb[:, jc * P + a : jc * P + a + 1],
                            rhs=z_t[:, jc * 512 + s * DOUT : jc * 512 + (s + 1) * DOUT],
                            start=(jc == 0),
                            stop=(jc == JC - 1),
                            tile_position=(0, 32 * s),
                            skip_group_check=True,
                        )
                # drain + C add (C pre-shuffled into this layout)
                nc.vector.tensor_add(
                    out_sb[:, b * DOUT : (b + 1) * DOUT],
                    pout[:, :DOUT],
                    C_shuf[:, b * DOUT : (b + 1) * DOUT],
                )

            def emit_out_dma(q):
                # groups 4q..4q+3 -> output rows 16q..16q+15
                for s in range(G4):
                    nc.sync.dma_start(
                        out_d[16 * q + s : 16 * (q + 1) : G4, :],
                        out_sb[32 * s : 32 * s + 1, 4 * q * DOUT : (4 * q + 4) * DOUT],
                    )

            # ---- main loop: X' two groups ahead, matvecs batched two at a
            # time LAG groups behind ----
            pending = []
            for b in range(NG):
                ybf = ypool.tile([P, JC * 512], bf, tag="ybf")
                if b == 0:
                    emit_mains_icp_major(b, x_tiles.pop(b), ybf)
                    pending.append((0, emit_z(ybf)))
                    x_tiles[2] = xpool.tile([P, IC * G4 * DOUT], f8, tag="X", name="x_t")
                    emit_x(2, x_tiles[2])
                else:
                    rev = b % 2 == 1
                    emit_mains(b, x_tiles.pop(b), ybf, rev)
                    if b + 2 < NG:
                        x_tiles[b + 2] = xpool.tile(
                            [P, IC * G4 * DOUT], f8, tag="X", name="x_t"
                        )
                        emit_x(b + 2, x_tiles[b + 2])
                    if b == 2:
                        emit_corrections()
                    if b % 2 == 1 and len(pending) > LAG:
                        emit_matvec(*pending.pop(0))
                        emit_matvec(*pending.pop(0))
                        if b == NG - 1:
                            emit_matvec(*pending.pop(0))
                    pending.append((b, emit_z(ybf, rev)))
                if b >= 6 and (b - 6) % 4 == 0:
                    emit_out_dma((b - 6) // 4)
            while pending:
                emit_matvec(*pending.pop(0))
            q = NG // 4 - 1
            nc.sync.dma_start(
                out_d[16 * q : 16 * (q + 1), :].rearrange(
                    "(t s) d -> s t d", s=G4
                ),
                out_sb[0:P:32, 4 * q * DOUT : (4 * q + 4) * DOUT].rearrange(
                    "s (t d) -> s t d", t=4
                ),
            )

    nc.compile()
    return nc


def _prep_inputs(inputs):
    """Host-side sharding + layout prep. Returns per-core input maps.

    Every tensor is pre-arranged to [128, rowbytes]: SBUF[p, chunk*W + x]
    = src[chunk*128 + p, x], so each DMA is 128 contiguous descriptors.
    wf = node_features @ weight is computed here (replicated, tiny).
    """
    import ml_dtypes

    bf16 = ml_dtypes.bfloat16
    f8 = ml_dtypes.float8_e4m3

    def fold(x, width):  # [C*128, W] -> [128, C*W]
        c = x.shape[0] // P
        assert x.shape == (c * P, width)
        return np.ascontiguousarray(
            x.reshape(c, P, width).transpose(1, 0, 2).reshape(P, c * width)
        )

    nf = np.asarray(inputs["node_features"], dtype=np.float32)
    adj = np.asarray(inputs["adjacency_matrix"], dtype=np.float32)
    mf = np.asarray(inputs["mask_father"], dtype=np.float32)[:, 0, :]
    ncnt = np.asarray(inputs["neighbor_count"], dtype=np.float32)
    mh = np.asarray(inputs["mask_hadamard"], dtype=np.float32)[:, 0, :]
    w = np.asarray(inputs["weight"], dtype=np.float32)

    mh8 = fold(mh - np.float32(0.5), N).astype(f8)
    mfs = mf / (ncnt * ncnt)  # fold 1/ncnt^2 into the father mask
    wf = nf @ w  # replicated, tiny: computed host-side per the sharding hint
    wfold = fold(wf, DOUT)
    wfb = wfold.astype(bf16)
    wf4 = np.ascontiguousarray(
        np.broadcast_to(
            wfold.reshape(P, JC, 1, DOUT), (P, JC, G4, DOUT)
        ).reshape(P, JC * 512)
    ).astype(bf16)
    in_maps = []
    for c in range(NCORES):
        rows = slice(c * ROWS, (c + 1) * ROWS)
        in_maps.append(
            {
                "mh8": mh8,
                "adjTc": (
                    fold(np.ascontiguousarray(adj[rows].T), ROWS) - np.float32(0.5)
                ).astype(bf16),
                "mfT": fold(np.ascontiguousarray(mfs[rows].T), ROWS).astype(bf16),
                "wfb": wfb,
                "wf4": wf4,
            }
        )
    return in_maps


def _run(inputs, trace=False):
    from concourse import bass_utils

    if "nc" not in _CACHE:
        _CACHE["nc"] = _build()
    nc = _CACHE["nc"]
    in_maps = _prep_inputs(inputs)
    res = bass_utils.run_bass_kernel_spmd(
        nc, in_maps, core_ids=list(range(NCORES)), trace=trace
    )
    out = np.concatenate([r["out"] for r in res.results], axis=0)
    return out, res


def kernel(**inputs):
    out, _ = _run(inputs, trace=False)
    return out
